# revision 15
# baseline (speedup 1.0000x reference)
"""Bidirectional 2-layer GRU encoder on 8 Trainium2 NeuronCores.

Reference quirk: the "backward" direction flips the FEATURE axis of each
timestep (not time), so all 4 GRU chains scan forward in time.  Layer-1
chains depend only on the same-direction layer-0 chain, so:

  core c (of 8): direction d = c // 4 (0=fwd, 1=bwd), batch quarter q = c % 4
  Each core runs its layer-0 chain then layer-1 chain for 16 batch rows,
  with zero cross-core communication.  The feature flip is folded into the
  backward layer-0 input weights on the host.

Device layout ("option T", weight-stationary):
  All tensors keep hidden/gate units on SBUF partitions, batch on the free
  axis.  Recurrent matmul: out[gate_tile, batch] = WhhT_tile.T @ hT_tile.
  Gate pre-activations land in PSUM as [128, 12, BQ] (12 m-tiles of 128
  gate units x 16 batch); the GRU pointwise math runs on Vector/Scalar
  engines in the same layout, producing the new hidden state directly in
  the [unit, batch] layout the next matmul consumes.  No transposes on the
  device at all (x is pre-transposed on the host).

  Time is processed in chunks of C steps; the layer-0 input projection
  gi0 = x @ Wih0.T (+bias) is done per-chunk as a batched matmul, layer-1's
  gi1 = h0 @ Wih1.T likewise from the on-chip h0 chunk buffer.
"""

import numpy as np

import concourse.bass as bass
import concourse.mybir as mybir
from concourse import bacc
from concourse.bass import ds
from concourse.tile import TileContext
from concourse.bass_utils import run_bass_kernel_spmd

F32 = mybir.dt.float32
BF16 = mybir.dt.bfloat16
FP16 = mybir.dt.float16
AF = mybir.ActivationFunctionType
ALU = mybir.AluOpType

B, T, I, H = 64, 512, 256, 512
NCORES = 8
BQ = B // 4            # batch rows per core = 16
G3 = 3 * H             # 1536 gate columns
MT = G3 // 128         # 12 gate m-tiles
KH = H // 128          # 4 hidden k-tiles
KI = I // 128          # 2 input k-tiles

LAST_RESULTS = None    # set by _run for test introspection
C0_COLS = 32 * BQ      # output lag: chunk x lands at slot x+1


def build_gru_kernel(t_steps=T, chunk=32, mm_dt=F32, loop="fori", nodep=False):
    """One SPMD program; per-core data differences come via in_maps."""
    C = chunk
    CC = C * BQ                       # chunk columns
    n_cols = t_steps * BQ
    assert t_steps % C == 0

    n_iter = t_steps // C + 1
    pad_cols = n_iter * C * BQ
    nc = bacc.Bacc()
    xT = nc.dram_tensor("xT", [I, pad_cols], F32, kind="ExternalInput")
    w0i = nc.dram_tensor("w0i", [I, G3], mm_dt, kind="ExternalInput")
    w0h = nc.dram_tensor("w0h", [H, G3], mm_dt, kind="ExternalInput")
    w1i = nc.dram_tensor("w1i", [H, G3], mm_dt, kind="ExternalInput")
    w1h = nc.dram_tensor("w1h", [H, G3], mm_dt, kind="ExternalInput")
    bg0 = nc.dram_tensor("bg0", [128, MT], F32, kind="ExternalInput")
    bn0 = nc.dram_tensor("bn0", [128, KH * BQ], F32, kind="ExternalInput")
    bg1 = nc.dram_tensor("bg1", [128, MT], F32, kind="ExternalInput")
    bn1 = nc.dram_tensor("bn1", [128, KH * BQ], F32, kind="ExternalInput")
    outT = nc.dram_tensor("outT", [H, pad_cols], F32, kind="ExternalOutput")

    xT_r = xT[:, :].rearrange("(k p) n -> p k n", p=128)
    outT_r = outT[:, :].rearrange("(k p) n -> p k n", p=128)

    with TileContext(nc) as tc:
        with (
            tc.tile_pool(name="wts", bufs=1) as wts,
            tc.tile_pool(name="state", bufs=1) as state,
            tc.tile_pool(name="xc", bufs=2) as xcp,
            tc.tile_pool(name="tmp", bufs=2) as tmp,
            tc.tile_pool(name="pj", bufs=2, space="PSUM") as pj,
            tc.tile_pool(name="gate", bufs=2, space="PSUM") as gatep,
        ):
            # --- persistent tiles -------------------------------------
            w0i_s = wts.tile([128, KI, G3], mm_dt)
            w0h_s = wts.tile([128, KH, G3], mm_dt)
            w1i_s = wts.tile([128, KH, G3], mm_dt)
            w1h_s = wts.tile([128, KH, G3], mm_dt)
            bg0_s = wts.tile([128, MT], F32)
            bn0_s = wts.tile([128, KH, BQ], F32)
            bg1_s = wts.tile([128, MT], F32)
            bn1_s = wts.tile([128, KH, BQ], F32)
            for dst, src in (
                (w0i_s, w0i), (w0h_s, w0h), (w1i_s, w1i), (w1h_s, w1h),
            ):
                nc.sync.dma_start(
                    dst, src[:, :].rearrange("(k p) m -> p k m", p=128))
            for dst, src in (
                (bg0_s, bg0), (bg1_s, bg1),
            ):
                nc.sync.dma_start(dst, src[:, :])
            nc.sync.dma_start(
                bn0_s, bn0[:, :].rearrange("p (k b) -> p k b", k=KH))
            nc.sync.dma_start(
                bn1_s, bn1[:, :].rearrange("p (k b) -> p k b", k=KH))

            G01 = state.tile([128, 2, MT, CC], F32)  # gi chunks, both layers
            HO = state.tile([128, 2, KH, CC], mm_dt)  # h chunks: [*,0]=L0, [*,1]=L1
            G0, G1 = G01[:, 0], G01[:, 1]
            H0, O = HO[:, 0], HO[:, 1]
            bn01_s = wts.tile([128, 2, KH, BQ], F32)
            nc.vector.tensor_copy(bn01_s[:, 0], bn0_s)
            nc.vector.tensor_copy(bn01_s[:, 1], bn1_s)
            nc.vector.memset(G01, 0.0)
            nc.vector.memset(HO, 0.0)

            def projection(G, src, n_ktiles, wi_s, bg_s):
                """G[:, m, :] = wi.T @ src + bias."""
                for m in range(MT):
                    for n2 in range(CC // 512):
                        ps = pj.tile([128, 512], F32, tag="pj")
                        nsl = ds(n2 * 512, 512)
                        for k in range(n_ktiles):
                            nc.tensor.matmul(
                                ps, wi_s[:, k, m * 128:(m + 1) * 128],
                                src[:, k, nsl],
                                start=(k == 0), stop=(k == n_ktiles - 1))
                        nc.vector.tensor_scalar_add(
                            G[:, m, nsl], ps, bg_s[:, m:m + 1])

            def gru_mm_into(P2, ch, wh_s, Hbuf, t):
                prev = t if nodep else (t - 1) % C
                hprev = Hbuf[:, :, prev * BQ:(prev + 1) * BQ]
                for m in range(MT):
                    for k in range(KH):
                        nc.tensor.matmul(
                            P2[:, ch, m, :], wh_s[:, k, m * 128:(m + 1) * 128],
                            hprev[:, k, :],
                            start=(k == 0), stop=(k == KH - 1))
                return hprev

            def gru_step(G, wh_s, Hbuf, bn_s, t, lab):
                gsl = ds(t * BQ, BQ)
                P2 = gatep.tile([128, 2, MT, BQ], F32, tag="p2")
                hprev = gru_mm_into(P2, 0, wh_s, Hbuf, t)
                P = P2[:, 0]

                def mk(tag, g=KH):
                    return tmp.tile([128, g, BQ], F32, tag=lab + tag,
                                    name=lab + tag)

                rz, sg = mk("rz", 8), mk("sg", 8)
                u, t1, t2, nt, dd, ee = (mk(x) for x in
                                         ("u", "t1", "t2", "nt", "dd", "ee"))
                nc.vector.tensor_add(rz, P[:, 0:8, :], G[:, 0:8, gsl])
                nc.vector.tensor_add(u, P[:, 8:12, :], bn_s)
                nc.scalar.activation(sg, rz, AF.Sigmoid)
                nc.vector.tensor_mul(t1, sg[:, 0:4, :], u)
                nc.vector.tensor_add(t2, t1, G[:, 8:12, gsl])
                nc.scalar.activation(nt, t2, AF.Tanh)
                nc.vector.tensor_sub(dd, hprev, nt)
                nc.vector.tensor_mul(ee, sg[:, 4:8, :], dd)
                nc.vector.tensor_add(Hbuf[:, :, gsl], nt, ee)

            def gru_pair(t):
                """One L0 step + one L1 step with the two chains' pointwise
                math fused into single wide ops (halves instruction count —
                per-instruction sync overhead dominates the chain)."""
                gsl = ds(t * BQ, BQ)
                prev = t if nodep else (t - 1) % C
                P2 = gatep.tile([128, 2, MT, BQ], F32, tag="p2")
                gru_mm_into(P2, 0, w0h_s, H0, t)
                gru_mm_into(P2, 1, w1h_s, O, t)
                hprev = HO[:, :, :, prev * BQ:(prev + 1) * BQ]

                def mk(tag, g=KH):
                    return tmp.tile([128, 2, g, BQ], F32, tag=tag, name=tag)

                rz, sg = mk("rz", 8), mk("sg", 8)
                u, t1, t2, nt, dd, ee = (mk(x) for x in
                                         ("u", "t1", "t2", "nt", "dd", "ee"))
                nc.vector.tensor_add(rz, P2[:, :, 0:8, :], G01[:, :, 0:8, gsl])
                nc.vector.tensor_add(u, P2[:, :, 8:12, :], bn01_s)
                nc.scalar.activation(sg, rz, AF.Sigmoid)
                nc.vector.tensor_mul(t1, sg[:, :, 0:4, :], u)
                nc.vector.tensor_add(t2, t1, G01[:, :, 8:12, gsl])
                nc.scalar.activation(nt, t2, AF.Tanh)
                nc.vector.tensor_sub(dd, hprev, nt)
                nc.vector.tensor_mul(ee, sg[:, :, 4:8, :], dd)
                nc.vector.tensor_add(HO[:, :, :, gsl], nt, ee)

            def chunk_body(col0, do_l1=True):
                if do_l1:
                    # gi1 for the PREVIOUS chunk (H0 still holds its h0)
                    projection(G1, H0, KH, w1i_s, bg1_s)
                xc = xcp.tile([128, KI, CC], mm_dt, tag="xc")
                if mm_dt == F32:
                    nc.sync.dma_start(xc, xT_r[:, :, ds(col0, CC)])
                else:
                    nc.gpsimd.dma_start(xc, xT_r[:, :, ds(col0, CC)])
                projection(G0, xc, KI, w0i_s, bg0_s)
                for t in range(C):
                    if do_l1:
                        gru_pair(t)
                    else:
                        gru_step(G0, w0h_s, H0, bn0_s, t, "a")
                if do_l1:
                    if mm_dt == F32:
                        nc.sync.dma_start(outT_r[:, :, ds(col0, CC)], O)
                    else:
                        nc.gpsimd.dma_start(outT_r[:, :, ds(col0, CC)], O)

            # prologue: chunk 0 layer-0 only (keeps h1 initial state zero)
            chunk_body(0, do_l1=False)
            if loop == "fori":
                with tc.For_i(CC, n_iter * CC, CC) as col0:
                    chunk_body(col0)
            else:
                for col0 in range(CC, n_iter * CC, CC):
                    chunk_body(col0)

    nc.compile()
    return nc


def _prep_inputs(inputs, t_steps, mm_np):  # noqa: C901
    """Host-side shard/layout prep.  Returns per-core in_maps."""
    x = np.ascontiguousarray(np.asarray(inputs["x"], dtype=np.float32))

    def wset(d):
        sfx = "f" if d == 0 else "b"
        Wih0 = np.asarray(inputs[f"Wih_{sfx}0"], np.float32)
        if d == 1:
            Wih0 = Wih0[:, ::-1]          # fold the feature flip
        Whh0 = np.asarray(inputs[f"Whh_{sfx}0"], np.float32)
        Wih1 = np.asarray(inputs[f"Wih_{sfx}1"], np.float32)
        Whh1 = np.asarray(inputs[f"Whh_{sfx}1"], np.float32)
        bih0 = np.asarray(inputs[f"bih_{sfx}0"], np.float32)
        bhh0 = np.asarray(inputs[f"bhh_{sfx}0"], np.float32)
        bih1 = np.asarray(inputs[f"bih_{sfx}1"], np.float32)
        bhh1 = np.asarray(inputs[f"bhh_{sfx}1"], np.float32)

        def fold(bih, bhh):
            bg = bih.copy()
            bg[:2 * H] += bhh[:2 * H]                 # r, z: both biases
            return bg.reshape(MT, 128).T.copy()       # [128, 12]

        return {
            "w0i": np.ascontiguousarray(Wih0.T, dtype=mm_np),
            "w0h": np.ascontiguousarray(Whh0.T, dtype=mm_np),
            "w1i": np.ascontiguousarray(Wih1.T, dtype=mm_np),
            "w1h": np.ascontiguousarray(Whh1.T, dtype=mm_np),
            "bg0": fold(bih0, bhh0),
            "bn0": np.repeat(
                bhh0[2 * H:].reshape(KH, 128).T[:, :, None], BQ,
                axis=2).reshape(128, KH * BQ).copy(),
            "bg1": fold(bih1, bhh1),
            "bn1": np.repeat(
                bhh1[2 * H:].reshape(KH, 128).T[:, :, None], BQ,
                axis=2).reshape(128, KH * BQ).copy(),
        }

    wsets = [wset(0), wset(1)]
    in_maps = []
    for c in range(NCORES):
        d, q = c // 4, c % 4
        xq = x[q * BQ:(q + 1) * BQ, :t_steps]                # [16, T, 256]
        xT_c = xq.transpose(2, 1, 0).reshape(I, t_steps * BQ)  # col = t*16+b
        pad = np.zeros((I, chunk_cols(t_steps)), np.float32)
        pad[:, :t_steps * BQ] = xT_c
        m = {"xT": pad}
        m.update(wsets[d])
        in_maps.append(m)
    return in_maps


def _assemble(results, t_steps):
    out = np.empty((B, t_steps, 2 * H), dtype=np.float32)
    for c in range(NCORES):
        d, q = c // 4, c % 4
        oT = results[c]["outT"][:, C0_COLS:C0_COLS + t_steps * BQ]
        o = oT.reshape(H, t_steps, BQ).transpose(2, 1, 0)    # [16, T, 512]
        out[q * BQ:(q + 1) * BQ, :, d * H:(d + 1) * H] = o
    return out, out[:, -1, :].copy()


def chunk_cols(t_steps, C=32):
    return (t_steps // C + 1) * C * BQ


_NC_CACHE = {}


def _run(inputs, t_steps=T, chunk=32, mm_dt_name="f16", trace=False):
    global LAST_RESULTS
    if mm_dt_name == "f32":
        mm_dt, mm_np = F32, np.float32
    elif mm_dt_name == "f16":
        mm_dt, mm_np = FP16, np.float16
    else:
        import ml_dtypes
        mm_dt, mm_np = BF16, ml_dtypes.bfloat16
    key = (t_steps, chunk, mm_dt_name)
    if key not in _NC_CACHE:
        _NC_CACHE[key] = build_gru_kernel(t_steps, chunk, mm_dt)
    nc = _NC_CACHE[key]
    in_maps = _prep_inputs(inputs, t_steps, mm_np)
    res = run_bass_kernel_spmd(
        nc, in_maps, core_ids=list(range(NCORES)), trace=trace)
    LAST_RESULTS = res
    return _assemble(res.results, t_steps)


def kernel(**inputs):
    return _run(inputs)


if __name__ == "__main__":
    rng = np.random.default_rng(0)
    tt = 64
    fake = {"x": rng.standard_normal((B, tt, I), dtype=np.float32) * 0.5}
    for d in ("f", "b"):
        for j, in_sz in enumerate((I, H)):
            fake[f"Wih_{d}{j}"] = (rng.standard_normal((G3, in_sz), dtype=np.float32) * 0.05)
            fake[f"Whh_{d}{j}"] = (rng.standard_normal((G3, H), dtype=np.float32) * 0.05)
            fake[f"bih_{d}{j}"] = (rng.standard_normal((G3,), dtype=np.float32) * 0.05)
            fake[f"bhh_{d}{j}"] = (rng.standard_normal((G3,), dtype=np.float32) * 0.05)

    def np_cell(xt, h, Wih, Whh, bih, bhh):
        gi = xt @ Wih.T + bih
        gh = h @ Whh.T + bhh
        ir, iz, inn = np.split(gi, 3, 1)
        hr, hz, hn = np.split(gh, 3, 1)
        r = 1 / (1 + np.exp(-(ir + hr)))
        z = 1 / (1 + np.exp(-(iz + hz)))
        n = np.tanh(inn + r * hn)
        return (1 - z) * n + z * h

    def np_ref(x, **w):
        b = x.shape[0]
        hf0 = hf1 = hb0 = hb1 = np.zeros((b, H), np.float32)
        ys = []
        for t in range(x.shape[1]):
            xt = x[:, t]
            xb = xt[:, ::-1]
            hf0 = np_cell(xt, hf0, w["Wih_f0"], w["Whh_f0"], w["bih_f0"], w["bhh_f0"])
            hb0 = np_cell(xb, hb0, w["Wih_b0"], w["Whh_b0"], w["bih_b0"], w["bhh_b0"])
            hf1 = np_cell(hf0, hf1, w["Wih_f1"], w["Whh_f1"], w["bih_f1"], w["bhh_f1"])
            hb1 = np_cell(hb0, hb1, w["Wih_b1"], w["Whh_b1"], w["bih_b1"], w["bhh_b1"])
            ys.append(np.concatenate([hf1, hb1], 1))
        out = np.stack(ys, 1)
        return out, out[:, -1, :]

    exp, exp_last = np_ref(**fake)
    got, got_last = _run(fake, t_steps=tt)
    err = np.linalg.norm(got - exp) / np.linalg.norm(exp)
    print("rel l2 err:", err)
    print("max abs err:", np.abs(got - exp).max())
    assert err < 1e-5, "MISMATCH"
    print("KERNEL SMOKE OK")


# revision 17
# speedup vs baseline: 2.1632x; 2.1632x over previous
"""Bidirectional 2-layer GRU encoder on 8 Trainium2 NeuronCores.

Reference quirk: the "backward" direction flips the FEATURE axis of each
timestep (not time), so all 4 GRU chains scan forward in time.  Layer-1
chains depend only on the same-direction layer-0 chain, so:

  core c (of 8): direction d = c // 4 (0=fwd, 1=bwd), batch quarter q = c % 4
  Each core runs its layer-0 chain then layer-1 chain for 16 batch rows,
  with zero cross-core communication.  The feature flip is folded into the
  backward layer-0 input weights on the host.

Device layout ("option T", weight-stationary):
  All tensors keep hidden/gate units on SBUF partitions, batch on the free
  axis.  Recurrent matmul: out[gate_tile, batch] = WhhT_tile.T @ hT_tile.
  Gate pre-activations land in PSUM as [128, 12, BQ] (12 m-tiles of 128
  gate units x 16 batch); the GRU pointwise math runs on Vector/Scalar
  engines in the same layout, producing the new hidden state directly in
  the [unit, batch] layout the next matmul consumes.  No transposes on the
  device at all (x is pre-transposed on the host).

  Time is processed in chunks of C steps; the layer-0 input projection
  gi0 = x @ Wih0.T (+bias) is done per-chunk as a batched matmul, layer-1's
  gi1 = h0 @ Wih1.T likewise from the on-chip h0 chunk buffer.
"""

import numpy as np

import concourse.bass as bass
import concourse.mybir as mybir
from concourse import bacc
from concourse.bass import ds
from concourse.tile import TileContext
from concourse.bass_utils import run_bass_kernel_spmd

F32 = mybir.dt.float32
BF16 = mybir.dt.bfloat16
FP16 = mybir.dt.float16
AF = mybir.ActivationFunctionType
ALU = mybir.AluOpType

B, T, I, H = 64, 512, 256, 512
NCORES = 8
BQ = B // 4            # batch rows per core = 16
G3 = 3 * H             # 1536 gate columns
MT = G3 // 128         # 12 gate m-tiles
KH = H // 128          # 4 hidden k-tiles
KI = I // 128          # 2 input k-tiles

LAST_RESULTS = None    # set by _run for test introspection


def build_gru_kernel(t_steps=T, chunk=32, mm_dt=F32, loop="fori", nodep=False,
                     gi_dt=None):
    """One SPMD program; per-core data differences come via in_maps."""
    C = chunk
    CC = C * BQ                       # chunk columns
    if gi_dt is None:
        gi_dt = F32 if mm_dt == F32 else mm_dt
    n_cols = t_steps * BQ
    assert t_steps % C == 0

    n_iter = t_steps // C + 1
    pad_cols = n_iter * C * BQ
    nc = bacc.Bacc()
    xT = nc.dram_tensor("xT", [I, pad_cols], F32, kind="ExternalInput")
    w0i = nc.dram_tensor("w0i", [I, G3], mm_dt, kind="ExternalInput")
    w0h = nc.dram_tensor("w0h", [H, G3], mm_dt, kind="ExternalInput")
    w1i = nc.dram_tensor("w1i", [H, G3], mm_dt, kind="ExternalInput")
    w1h = nc.dram_tensor("w1h", [H, G3], mm_dt, kind="ExternalInput")
    bg0 = nc.dram_tensor("bg0", [128, MT], F32, kind="ExternalInput")
    bn0 = nc.dram_tensor("bn0", [128, KH * BQ], F32, kind="ExternalInput")
    bg1 = nc.dram_tensor("bg1", [128, MT], F32, kind="ExternalInput")
    bn1 = nc.dram_tensor("bn1", [128, KH * BQ], F32, kind="ExternalInput")
    outT = nc.dram_tensor("outT", [H, pad_cols], F32, kind="ExternalOutput")

    xT_r = xT[:, :].rearrange("(k p) n -> p k n", p=128)
    outT_r = outT[:, :].rearrange("(k p) n -> p k n", p=128)

    with TileContext(nc) as tc:
        with (
            tc.tile_pool(name="wts", bufs=1) as wts,
            tc.tile_pool(name="state", bufs=1) as state,
            tc.tile_pool(name="xc", bufs=2) as xcp,
            tc.tile_pool(name="tmp", bufs=2) as tmp,
            tc.tile_pool(name="pj", bufs=2, space="PSUM") as pj,
            tc.tile_pool(name="gate", bufs=2, space="PSUM") as gatep,
        ):
            # --- persistent tiles -------------------------------------
            w0i_s = wts.tile([128, KI, G3], mm_dt)
            w0h_s = wts.tile([128, KH, G3], mm_dt)
            w1i_s = wts.tile([128, KH, G3], mm_dt)
            w1h_s = wts.tile([128, KH, G3], mm_dt)
            bg0_s = wts.tile([128, MT], F32)
            bn0_s = wts.tile([128, KH, BQ], F32)
            bg1_s = wts.tile([128, MT], F32)
            bn1_s = wts.tile([128, KH, BQ], F32)
            for dst, src in (
                (w0i_s, w0i), (w0h_s, w0h), (w1i_s, w1i), (w1h_s, w1h),
            ):
                nc.sync.dma_start(
                    dst, src[:, :].rearrange("(k p) m -> p k m", p=128))
            for dst, src in (
                (bg0_s, bg0), (bg1_s, bg1),
            ):
                nc.sync.dma_start(dst, src[:, :])
            nc.sync.dma_start(
                bn0_s, bn0[:, :].rearrange("p (k b) -> p k b", k=KH))
            nc.sync.dma_start(
                bn1_s, bn1[:, :].rearrange("p (k b) -> p k b", k=KH))

            G01 = state.tile([128, 2, MT, CC], gi_dt)  # gi chunks, both layers
            HO = state.tile([128, 2, KH, CC], mm_dt)  # h chunks: [*,0]=L0, [*,1]=L1
            G0, G1 = G01[:, 0], G01[:, 1]
            H0, O = HO[:, 0], HO[:, 1]
            bn01_s = wts.tile([128, 2, KH, BQ], F32)
            nc.vector.tensor_copy(bn01_s[:, 0], bn0_s)
            nc.vector.tensor_copy(bn01_s[:, 1], bn1_s)
            nc.vector.memset(G01, 0.0)
            nc.vector.memset(HO, 0.0)

            def projection(G, src, n_ktiles, wi_s, bg_s):
                """G[:, m, :] = wi.T @ src + bias."""
                for m in range(MT):
                    for n2 in range(CC // 512):
                        ps = pj.tile([128, 512], F32, tag="pj")
                        nsl = ds(n2 * 512, 512)
                        for k in range(n_ktiles):
                            nc.tensor.matmul(
                                ps, wi_s[:, k, m * 128:(m + 1) * 128],
                                src[:, k, nsl],
                                start=(k == 0), stop=(k == n_ktiles - 1))
                        nc.vector.tensor_scalar_add(
                            G[:, m, nsl], ps, bg_s[:, m:m + 1])

            def gru_mm_into(P2, ch, wh_s, Hbuf, t):
                prev = t if nodep else (t - 1) % C
                hprev = Hbuf[:, :, prev * BQ:(prev + 1) * BQ]
                for m in range(MT):
                    for k in range(KH):
                        nc.tensor.matmul(
                            P2[:, ch, m, :], wh_s[:, k, m * 128:(m + 1) * 128],
                            hprev[:, k, :],
                            start=(k == 0), stop=(k == KH - 1))
                return hprev

            def gru_step(G, wh_s, Hbuf, bn_s, t, lab):
                gsl = ds(t * BQ, BQ)
                P2 = gatep.tile([128, 2, MT, BQ], F32, tag="p2")
                hprev = gru_mm_into(P2, 0, wh_s, Hbuf, t)
                P = P2[:, 0]

                def mk(tag, g=KH):
                    return tmp.tile([128, g, BQ], F32, tag=lab + tag,
                                    name=lab + tag)

                rz, sg = mk("rz", 8), mk("sg", 8)
                u, t1, t2, nt, dd, ee = (mk(x) for x in
                                         ("u", "t1", "t2", "nt", "dd", "ee"))
                nc.vector.tensor_add(rz, P[:, 0:8, :], G[:, 0:8, gsl])
                nc.vector.tensor_add(u, P[:, 8:12, :], bn_s)
                nc.scalar.activation(sg, rz, AF.Sigmoid)
                nc.vector.tensor_mul(t1, sg[:, 0:4, :], u)
                nc.vector.tensor_add(t2, t1, G[:, 8:12, gsl])
                nc.scalar.activation(nt, t2, AF.Tanh)
                nc.vector.tensor_sub(dd, hprev, nt)
                nc.vector.tensor_mul(ee, sg[:, 4:8, :], dd)
                nc.vector.tensor_add(Hbuf[:, :, gsl], nt, ee)

            def gru_pair(t):
                """One L0 step + one L1 step with the two chains' pointwise
                math fused into single wide ops (halves instruction count —
                per-instruction sync overhead dominates the chain)."""
                gsl = ds(t * BQ, BQ)
                prev = t if nodep else (t - 1) % C
                P2 = gatep.tile([128, 2, MT, BQ], F32, tag="p2")
                gru_mm_into(P2, 0, w0h_s, H0, t)
                gru_mm_into(P2, 1, w1h_s, O, t)
                hprev = HO[:, :, :, prev * BQ:(prev + 1) * BQ]

                def mk(tag, g=KH):
                    return tmp.tile([128, 2, g, BQ], F32, tag=tag, name=tag)

                rz, sg = mk("rz", 8), mk("sg", 8)
                u, t1, t2, nt, dd, ee = (mk(x) for x in
                                         ("u", "t1", "t2", "nt", "dd", "ee"))
                nc.vector.tensor_add(rz, P2[:, :, 0:8, :], G01[:, :, 0:8, gsl])
                nc.vector.tensor_add(u, P2[:, :, 8:12, :], bn01_s)
                nc.scalar.activation(sg, rz, AF.Sigmoid)
                nc.vector.tensor_mul(t1, sg[:, :, 0:4, :], u)
                nc.vector.tensor_add(t2, t1, G01[:, :, 8:12, gsl])
                nc.scalar.activation(nt, t2, AF.Tanh)
                nc.vector.tensor_sub(dd, hprev, nt)
                nc.vector.tensor_mul(ee, sg[:, :, 4:8, :], dd)
                nc.vector.tensor_add(HO[:, :, :, gsl], nt, ee)

            def chunk_body(col0, do_l1=True):
                if do_l1:
                    # gi1 for the PREVIOUS chunk (H0 still holds its h0)
                    projection(G1, H0, KH, w1i_s, bg1_s)
                xc = xcp.tile([128, KI, CC], mm_dt, tag="xc")
                if mm_dt == F32:
                    nc.sync.dma_start(xc, xT_r[:, :, ds(col0, CC)])
                else:
                    nc.gpsimd.dma_start(xc, xT_r[:, :, ds(col0, CC)])
                projection(G0, xc, KI, w0i_s, bg0_s)
                for t in range(C):
                    if do_l1:
                        gru_pair(t)
                    else:
                        gru_step(G0, w0h_s, H0, bn0_s, t, "a")
                if do_l1:
                    if mm_dt == F32:
                        nc.sync.dma_start(outT_r[:, :, ds(col0, CC)], O)
                    else:
                        nc.gpsimd.dma_start(outT_r[:, :, ds(col0, CC)], O)

            # prologue: chunk 0 layer-0 only (keeps h1 initial state zero)
            chunk_body(0, do_l1=False)
            if loop == "fori":
                with tc.For_i(CC, n_iter * CC, CC) as col0:
                    chunk_body(col0)
            else:
                for col0 in range(CC, n_iter * CC, CC):
                    chunk_body(col0)

    nc.compile()
    return nc


def _prep_inputs(inputs, t_steps, mm_np, chunk=32):  # noqa: C901
    """Host-side shard/layout prep.  Returns per-core in_maps."""
    x = np.ascontiguousarray(np.asarray(inputs["x"], dtype=np.float32))

    def wset(d):
        sfx = "f" if d == 0 else "b"
        Wih0 = np.asarray(inputs[f"Wih_{sfx}0"], np.float32)
        if d == 1:
            Wih0 = Wih0[:, ::-1]          # fold the feature flip
        Whh0 = np.asarray(inputs[f"Whh_{sfx}0"], np.float32)
        Wih1 = np.asarray(inputs[f"Wih_{sfx}1"], np.float32)
        Whh1 = np.asarray(inputs[f"Whh_{sfx}1"], np.float32)
        bih0 = np.asarray(inputs[f"bih_{sfx}0"], np.float32)
        bhh0 = np.asarray(inputs[f"bhh_{sfx}0"], np.float32)
        bih1 = np.asarray(inputs[f"bih_{sfx}1"], np.float32)
        bhh1 = np.asarray(inputs[f"bhh_{sfx}1"], np.float32)

        def fold(bih, bhh):
            bg = bih.copy()
            bg[:2 * H] += bhh[:2 * H]                 # r, z: both biases
            return bg.reshape(MT, 128).T.copy()       # [128, 12]

        return {
            "w0i": np.ascontiguousarray(Wih0.T, dtype=mm_np),
            "w0h": np.ascontiguousarray(Whh0.T, dtype=mm_np),
            "w1i": np.ascontiguousarray(Wih1.T, dtype=mm_np),
            "w1h": np.ascontiguousarray(Whh1.T, dtype=mm_np),
            "bg0": fold(bih0, bhh0),
            "bn0": np.repeat(
                bhh0[2 * H:].reshape(KH, 128).T[:, :, None], BQ,
                axis=2).reshape(128, KH * BQ).copy(),
            "bg1": fold(bih1, bhh1),
            "bn1": np.repeat(
                bhh1[2 * H:].reshape(KH, 128).T[:, :, None], BQ,
                axis=2).reshape(128, KH * BQ).copy(),
        }

    wsets = [wset(0), wset(1)]
    in_maps = []
    for c in range(NCORES):
        d, q = c // 4, c % 4
        xq = x[q * BQ:(q + 1) * BQ, :t_steps]                # [16, T, 256]
        xT_c = xq.transpose(2, 1, 0).reshape(I, t_steps * BQ)  # col = t*16+b
        pad = np.zeros((I, chunk_cols(t_steps, chunk)), np.float32)
        pad[:, :t_steps * BQ] = xT_c
        m = {"xT": pad}
        m.update(wsets[d])
        in_maps.append(m)
    return in_maps


def _assemble(results, t_steps, chunk=32):
    c0_cols = chunk * BQ
    out = np.empty((B, t_steps, 2 * H), dtype=np.float32)
    for c in range(NCORES):
        d, q = c // 4, c % 4
        oT = results[c]["outT"][:, c0_cols:c0_cols + t_steps * BQ]
        o = oT.reshape(H, t_steps, BQ).transpose(2, 1, 0)    # [16, T, 512]
        out[q * BQ:(q + 1) * BQ, :, d * H:(d + 1) * H] = o
    return out, out[:, -1, :].copy()


def chunk_cols(t_steps, C=32):
    return (t_steps // C + 1) * C * BQ


_NC_CACHE = {}


def _run(inputs, t_steps=T, chunk=32, mm_dt_name="f16", trace=False):
    global LAST_RESULTS
    if mm_dt_name == "f32":
        mm_dt, mm_np = F32, np.float32
    elif mm_dt_name == "f16":
        mm_dt, mm_np = FP16, np.float16
    else:
        import ml_dtypes
        mm_dt, mm_np = BF16, ml_dtypes.bfloat16
    key = (t_steps, chunk, mm_dt_name)
    if key not in _NC_CACHE:
        _NC_CACHE[key] = build_gru_kernel(t_steps, chunk, mm_dt)
    nc = _NC_CACHE[key]
    in_maps = _prep_inputs(inputs, t_steps, mm_np, chunk)
    res = run_bass_kernel_spmd(
        nc, in_maps, core_ids=list(range(NCORES)), trace=trace)
    LAST_RESULTS = res
    return _assemble(res.results, t_steps, chunk)


def kernel(**inputs):
    return _run(inputs)


if __name__ == "__main__":
    rng = np.random.default_rng(0)
    tt = 64
    fake = {"x": rng.standard_normal((B, tt, I), dtype=np.float32) * 0.5}
    for d in ("f", "b"):
        for j, in_sz in enumerate((I, H)):
            fake[f"Wih_{d}{j}"] = (rng.standard_normal((G3, in_sz), dtype=np.float32) * 0.05)
            fake[f"Whh_{d}{j}"] = (rng.standard_normal((G3, H), dtype=np.float32) * 0.05)
            fake[f"bih_{d}{j}"] = (rng.standard_normal((G3,), dtype=np.float32) * 0.05)
            fake[f"bhh_{d}{j}"] = (rng.standard_normal((G3,), dtype=np.float32) * 0.05)

    def np_cell(xt, h, Wih, Whh, bih, bhh):
        gi = xt @ Wih.T + bih
        gh = h @ Whh.T + bhh
        ir, iz, inn = np.split(gi, 3, 1)
        hr, hz, hn = np.split(gh, 3, 1)
        r = 1 / (1 + np.exp(-(ir + hr)))
        z = 1 / (1 + np.exp(-(iz + hz)))
        n = np.tanh(inn + r * hn)
        return (1 - z) * n + z * h

    def np_ref(x, **w):
        b = x.shape[0]
        hf0 = hf1 = hb0 = hb1 = np.zeros((b, H), np.float32)
        ys = []
        for t in range(x.shape[1]):
            xt = x[:, t]
            xb = xt[:, ::-1]
            hf0 = np_cell(xt, hf0, w["Wih_f0"], w["Whh_f0"], w["bih_f0"], w["bhh_f0"])
            hb0 = np_cell(xb, hb0, w["Wih_b0"], w["Whh_b0"], w["bih_b0"], w["bhh_b0"])
            hf1 = np_cell(hf0, hf1, w["Wih_f1"], w["Whh_f1"], w["bih_f1"], w["bhh_f1"])
            hb1 = np_cell(hb0, hb1, w["Wih_b1"], w["Whh_b1"], w["bih_b1"], w["bhh_b1"])
            ys.append(np.concatenate([hf1, hb1], 1))
        out = np.stack(ys, 1)
        return out, out[:, -1, :]

    exp, exp_last = np_ref(**fake)
    got, got_last = _run(fake, t_steps=tt)
    err = np.linalg.norm(got - exp) / np.linalg.norm(exp)
    print("rel l2 err:", err)
    print("max abs err:", np.abs(got - exp).max())
    assert err < 2e-3, "MISMATCH"
    print("KERNEL SMOKE OK")
